# revision 51
# baseline (speedup 1.0000x reference)
"""TRN2 Bass kernel for nn_Attn_63230508532520.

reference:
    proj = history @ W.T + b            # [S1, N]
    energies = out_state @ proj.T       # [S2, S1]
    out = softmax(energies, axis=-1)

Math used here:
    energies = out_state @ W @ history.T + (out_state @ b) 1^T
    The bias term is constant per row -> softmax-invariant -> dropped.
    G = out_state @ W (per-core slice), scores = G @ history.T, row softmax.

Sharding: out_state rows (S2=4096) split across 8 cores (512 rows each);
W and history replicated. Host pre-transposes out_state slices and history.

Dtypes: everything is pre-quantized to fp16 on host (W 2MB, ost 1MB,
history 8MB per core instead of 4/2/16 as fp32) and G is evacuated from
PSUM as fp16, so all matmuls run fp16 x fp16 with fp32 PSUM accumulation
at the full 512-row/216ns PE cadence (fp16 stationary tiles also
double-buffer LDWEIGHTS where fp32r ones expose it; the BIR verifier
forbids mixing f32r with fp16 operands). Output fp16, host upcasts.
Deterministic rel err 0.0186 vs the 2e-2 gate (fixed-seed inputs; the
numpy quantization sim reproduces the HW value to 6 significant digits).

Per-core pipeline (~95.5us HW exec; ~7.5us is fixed engine boot before
the first DMA descriptor can even issue, Phase B runs at the PE roofline
with zero stall cycles):
  Warmup:  ~10 dummy matmuls on a GpSimd-memset tile keep the PE busy
           from its preamble (~7.5us) until real operands land (~13.8us)
           so its DVFS clock (0.65 -> 1.2 -> 2.4 GHz, needs ~9us of
           sustained execution) is mostly ramped for the real stream.
  Phase A: G.T [1024, 512] = W-stationary fp16 matmuls accumulated over
           n, PSUM evacuated to SBUF as fp16. ost is one [128, 4096]
           tile (panel n in cols n*512..), W four dual-panel [128, 2048]
           tiles interleaved with their consuming groups so each group
           gets its own DMA-completion semaphore (back-to-back issue
           coalesces the waits and stalls the PE until the last panel).
  Phase B: history.T lives fully resident in SBUF as 8 block-major fp16
           tiles [128, 4096] (t-block b holds all 8 feature panels side
           by side), loaded once behind the Phase-A operands. Score
           groups (s-chunk i, t-block b) run b-outer for b<4 (DMA-paced
           warmup), then per-i tails (i,4..7)+finalize(i) so each
           chunk's softmax finalize + store overlaps the next chunk's
           matmuls; only finalize(3) is exposed. Steady state: 216ns
           matmul cadence (the 512-row fp16 roofline), DVE block-max +
           ACT exp per group both well under the 1.73us group time.
  Finalize: flash-style deferred normalization per chunk: global max from
           the 8 block maxes, factors f_ij = exp(m_ij - M_i)/S_i, scale
           slices in place in the fp16 exp buffer (6 on DVE, 2 on ACT),
           stores streamed per-half (per-quarter for the last chunk, so
           the final exposed lone store transfer is only 0.25MB).
"""
import os
import numpy as np
from contextlib import ExitStack

S2, S1, N = 4096, 4096, 1024
NCORES = 8
SC = S2 // NCORES          # 512 rows per core
NB_T = S1 // 512           # 8 t-blocks
NB_M = N // 128            # 8 contraction chunks
NB_I = SC // 128           # 4 s-chunks per core

_CACHE = {}


def _build():
    import concourse.bacc as bacc
    import concourse.mybir as mybir
    import concourse.tile as tile

    F32 = mybir.dt.float32
    F16 = mybir.dt.float16

    nc = bacc.Bacc()
    # W: 4 dual-panel tiles [128, 2048] fp16; ost: one [128, 4096] fp16 tile
    w_r = nc.declare_dram_parameter("w_r", [N // 2, 2 * N], F16, isOutput=False)
    ost_r = nc.declare_dram_parameter("ost_r", [128, NB_M * SC], F16, isOutput=False)
    # block-major fp16 history: row b*128+p, col m*512+c = history[b*512+c, m*128+p]
    ht_r = nc.declare_dram_parameter("ht_r", [N, S1], F16, isOutput=False)
    probs = nc.declare_dram_parameter("probs", [SC, S1], F16, isOutput=True)

    with tile.TileContext(nc) as tc, ExitStack() as ctx:
        gt_pool = ctx.enter_context(tc.tile_pool(name="gt", bufs=1))
        htb_pool = ctx.enter_context(tc.tile_pool(name="htb", bufs=1))
        exp_pool = ctx.enter_context(tc.tile_pool(name="exp", bufs=1))
        small = ctx.enter_context(tc.tile_pool(name="small", bufs=1))
        ps = ctx.enter_context(tc.tile_pool(name="ps", bufs=8, space="PSUM"))

        # ---- PE warmup (results discarded): keeps the tensor engine busy
        # before the first operands land so its DVFS clock (0.65 -> 1.2 ->
        # 2.4 GHz, needs ~10us of sustained execution) is ramped for the
        # real stream. The memset runs on GpSimd, whose engine preamble
        # finishes earliest (~5.2us; the DVE path would stall it to ~8.4).
        warm = small.tile([128, 512], F16, tag="warm", name="warm")
        nc.gpsimd.memset(warm[:], 0.0)
        WARMUP_MM = 10
        for k in range(WARMUP_MM):
            pw = ps.tile([128, 512], F32, tag="ps")
            nc.tensor.matmul(pw[:], lhsT=warm[:, 0:128], rhs=warm[:],
                             start=True, stop=True)

        # ---- Phase A: G.T = (out_state_slice @ W).T, [m, s] layout ----
        gt = []
        with tc.tile_pool(name="win", bufs=1) as win:
            # ost is one [128, 4096] fp16 tile (panel n in cols n*512..):
            # a single 1MB 8KB-line DMA, pushed first. W is host-rearranged
            # panel-major fp16 in 4 dual-panel tiles [128, 2048] (dual j
            # holds panels 2j, 2j+1), so G.T group m depends only on dual
            # m//2 + ost; dual DMAs are interleaved with their consuming
            # groups so each gets its own DMA semaphore.
            # transfer completion time tracks total in-flight bytes, not
            # own size, so splitting ost buys nothing - one push each
            ostb = win.tile([128, NB_M * SC], F16, tag="ostb")
            nc.sync.dma_start(out=ostb, in_=ost_r[:, :])
            wd0 = win.tile([128, 2 * N], F16, tag="wd0")
            nc.sync.dma_start(out=wd0, in_=w_r[0:128, :])
            w_sb = [wd0]
            htb = []

            def issue_htb(b):
                t = htb_pool.tile([128, N * 4], F16, tag=f"htb{b}", name=f"htb{b}")
                nc.sync.dma_start(out=t, in_=ht_r[b * 128:(b + 1) * 128, :])
                htb.append(t)

            for m in range(NB_M):
                if m % 2 == 0 and m + 2 < NB_M:
                    j = m // 2 + 1
                    wt = win.tile([128, 2 * N], F16, tag=f"wd{j}")
                    nc.sync.dma_start(out=wt, in_=w_r[j * 128:(j + 1) * 128, :])
                    w_sb.append(wt)
                if m == NB_M - 2:
                    # history blocks queue behind all Phase-A operands
                    for b in range(NB_T):
                        issue_htb(b)
                wsl = w_sb[m // 2]
                woff = (m % 2) * N
                pg = ps.tile([128, SC], F32, tag="ps")
                for n in range(NB_M):
                    nc.tensor.matmul(
                        pg[:],
                        lhsT=wsl[:, woff + n * 128:woff + (n + 1) * 128],
                        rhs=ostb[:, n * 512:(n + 1) * 512],
                        start=(n == 0), stop=(n == NB_M - 1))
                g = gt_pool.tile([128, SC], F16, tag=f"gt{m}")
                nc.vector.tensor_copy(out=g[:], in_=pg[:])
                gt.append(g)

        # ---- Phase B: scores + streaming exp ----
        expb = [exp_pool.tile([128, S1], F16, tag=f"exp{i}", name=f"exp{i}")
                for i in range(NB_I)]
        # the LAST chunk's t-block 7 softmax pass runs as two 256-col
        # halves (slots 7, 8): its reduce+exp sit on the kernel's exposed
        # tail chain, and halving them shortens it ~0.7us. Other chunks'
        # finalize work is covered by later matmuls, so they keep the
        # cheaper single-pass form (slot 8 unused).
        NSL = NB_T + 1
        LAST = NB_I - 1
        nmax = [small.tile([128, NSL], F32, tag=f"nmax{i}", name=f"nmax{i}")
                for i in range(NB_I)]
        ssum = [small.tile([128, NSL], F32, tag=f"ssum{i}", name=f"ssum{i}")
                for i in range(NB_I)]

        def softpass(i, sl_idx, pscore, psl, esl):
            nc.vector.tensor_reduce(out=nmax[i][:, sl_idx:sl_idx + 1],
                                    in_=pscore[:, psl],
                                    axis=mybir.AxisListType.X,
                                    op=mybir.AluOpType.max, negate=True)
            nc.scalar.activation(out=expb[i][:, esl], in_=pscore[:, psl],
                                 func=mybir.ActivationFunctionType.Exp,
                                 bias=nmax[i][:, sl_idx:sl_idx + 1], scale=1.0,
                                 accum_out=ssum[i][:, sl_idx:sl_idx + 1])

        def group(i, b):
            pscore = ps.tile([128, 512], F32, tag="ps")
            for m in range(NB_M):
                nc.tensor.matmul(pscore[:],
                                 lhsT=gt[m][:, i * 128:(i + 1) * 128],
                                 rhs=htb[b][:, m * 512:(m + 1) * 512],
                                 start=(m == 0), stop=(m == NB_M - 1))
            if i == LAST and b == NB_T - 1:
                softpass(i, b, pscore, slice(0, 256),
                         slice(b * 512, b * 512 + 256))
                softpass(i, b + 1, pscore, slice(256, 512),
                         slice(b * 512 + 256, (b + 1) * 512))
            else:
                softpass(i, b, pscore, slice(0, 512),
                         slice(b * 512, (b + 1) * 512))

        def finalize(i):
            """Global max over block maxes, rescale factors, scale+store.

            nmax holds nm_ij = -m_ij; NM_i = min_j nm_ij = -M_i, so
            e_ij = exp(m_ij - M_i) = exp(-nm_ij + NM_i) = Exp(scale=-1, bias=NM_i).
            """
            nsl = NSL if i == LAST else NB_T
            nm = small.tile([128, 1], F32, tag=f"nm{i}", name=f"nm{i}")
            nc.vector.tensor_reduce(out=nm[:], in_=nmax[i][:, 0:nsl],
                                    axis=mybir.AxisListType.X,
                                    op=mybir.AluOpType.min)
            e = small.tile([128, NSL], F32, tag=f"e{i}", name=f"e{i}")
            nc.scalar.activation(out=e[:, 0:nsl], in_=nmax[i][:, 0:nsl],
                                 func=mybir.ActivationFunctionType.Exp,
                                 bias=nm[:], scale=-1.0)
            wsum = small.tile([128, NSL], F32, tag=f"ws{i}", name=f"ws{i}")
            nc.vector.tensor_mul(wsum[:, 0:nsl], e[:, 0:nsl], ssum[i][:, 0:nsl])
            s = small.tile([128, 1], F32, tag=f"s{i}", name=f"s{i}")
            nc.vector.tensor_reduce(out=s[:], in_=wsum[:, 0:nsl],
                                    axis=mybir.AxisListType.X,
                                    op=mybir.AluOpType.add)
            r = small.tile([128, 1], F32, tag=f"r{i}", name=f"r{i}")
            nc.vector.reciprocal(out=r[:], in_=s[:])
            f = small.tile([128, NSL], F32, tag=f"f{i}", name=f"f{i}")
            nc.vector.tensor_scalar_mul(f[:, 0:nsl], e[:, 0:nsl], r[:])
            # scale slot -> expb column range; for the last chunk slots
            # 7, 8 are the two halves of t-block 7
            if i == LAST:
                slots = [(b, slice(b * 512, (b + 1) * 512))
                         for b in range(NB_T - 1)]
                slots.append((7, slice(3584, 3840)))
                slots.append((8, slice(3840, 4096)))
            else:
                slots = [(b, slice(b * 512, (b + 1) * 512))
                         for b in range(NB_T)]
            for k, (sid, sl) in enumerate(slots):
                # DVE does fp16 scale slices in ~345ns vs ~800ns on ACT
                # (which is also busy with the exps); GpSimd is 22x slower
                # at TENSOR_SCALAR (~7.5us ucode launch) - never use it.
                if k < 6:
                    nc.vector.tensor_scalar_mul(expb[i][:, sl], expb[i][:, sl],
                                                f[:, sid:sid + 1])
                else:
                    nc.scalar.mul(expb[i][:, sl], expb[i][:, sl],
                                  f[:, sid:sid + 1])
                # stores stream out as their blocks are scaled; the last
                # chunk uses quarters so the final exposed transfer is only
                # 0.25MB (a lone store transfer runs well below aggregate
                # DMA bandwidth, so its size sets the kernel's tail; each
                # push also costs ~0.7us of Sync time, so no finer).
                if i == NB_I - 1:
                    if k in (1, 3, 5):
                        st = (k - 1) * 512
                        nc.sync.dma_start(
                            out=probs[i * 128:(i + 1) * 128, st:st + 1024],
                            in_=expb[i][:, st:st + 1024])
                elif k == 3:
                    nc.sync.dma_start(out=probs[i * 128:(i + 1) * 128, 0:2048],
                                      in_=expb[i][:, 0:2048])
            if i == NB_I - 1:
                # the kernel's last exposed transfer: split across the
                # Sync and ACT DMA queues - the ACT push issues right
                # after ACT finishes that very data's scale (no Sync
                # queue backlog in front of it) and the two 128KB halves
                # transfer in parallel instead of one lone 0.25MB store
                # at ~95GB/s. (GpSimd-issued stores push ~0.6us later -
                # its DGE prep is slower; measured, don't use.)
                nc.sync.dma_start(out=probs[i * 128:(i + 1) * 128, 3072:3584],
                                  in_=expb[i][:, 3072:3584])
                nc.scalar.dma_start(out=probs[i * 128:(i + 1) * 128, 3584:4096],
                                    in_=expb[i][:, 3584:4096])
            else:
                nc.sync.dma_start(out=probs[i * 128:(i + 1) * 128, 2048:4096],
                                  in_=expb[i][:, 2048:4096])

        # b-outer warmup while history blocks stream in, then per-chunk
        # tails (4 score groups + finalize) so each chunk's softmax
        # finalize overlaps the next chunk's matmuls.
        TAIL = 4
        for b in range(NB_T - TAIL):
            for i in range(NB_I):
                group(i, b)
        for i in range(NB_I):
            for b in range(NB_T - TAIL, NB_T):
                group(i, b)
            finalize(i)

    nc.finalize()
    return nc


def _get_nc():
    if "nc" not in _CACHE:
        _CACHE["nc"] = _build()
    return _CACHE["nc"]


def kernel(out_state, history, W, b):
    from concourse.bass_utils import run_bass_kernel_spmd

    out_state = np.ascontiguousarray(out_state, dtype=np.float32)
    history = np.ascontiguousarray(history, dtype=np.float32)
    W = np.ascontiguousarray(W, dtype=np.float32)
    # panel-major: Wp[m*128+p, n*128+c] = W[n*128+p, m*128+c], then packed
    # into 4 dual-panel fp16 tiles wd[j][p, k*1024 + nc] = Wp[(2j+k)*128+p, nc]
    Wp = W.reshape(8, 128, 8, 128).transpose(2, 1, 0, 3).reshape(1024, 1024)
    wd = np.ascontiguousarray(
        Wp.reshape(4, 2, 128, 1024).transpose(0, 2, 1, 3)
        .reshape(512, 2048).astype(np.float16))

    # block-major fp16 history.T: ht[b*128+p, m*512+c] = history[b*512+c, m*128+p]
    ht = np.ascontiguousarray(
        history.T.reshape(8, 128, 8, 512).transpose(2, 1, 0, 3)
        .reshape(1024, 4096).astype(np.float16))
    in_maps = []
    for c in range(NCORES):
        # one [128, 4096] fp16 tile: ostb[p, n*512+s] = out_state[c*SC+s, n*128+p]
        ostb = np.ascontiguousarray(
            out_state[c * SC:(c + 1) * SC, :].T
            .reshape(8, 128, 512).transpose(1, 0, 2)
            .reshape(128, 4096).astype(np.float16))
        in_maps.append({"w_r": wd, "ost_r": ostb, "ht_r": ht})

    nc = _get_nc()
    trace = bool(int(os.environ.get("KERNEL_TRACE", "0")))
    res = run_bass_kernel_spmd(nc, in_maps, list(range(NCORES)), trace=trace)
    _CACHE["last_result"] = res
    out = np.empty((S2, S1), dtype=np.float32)
    for c in range(NCORES):
        out[c * SC:(c + 1) * SC, :] = res.results[c]["probs"].astype(np.float32)
    return out
